# revision 14
# baseline (speedup 1.0000x reference)
"""Trainium2 Bass kernel for channel ("transposed") attention:
  qkv = conv3x3(conv1x1(x)); per-head L2-normalized channel attention; 1x1 proj.

Sharding: pure data-parallel — batch 8 across 8 NeuronCores (one image each).
Per-core pipeline (all matmuls bf16 with f32 PSUM accumulation):
  A: y1p = w1 @ xp (host-padded input, 130x130) -> DRAM (bf16)
  B: conv3x3 as 9 shifted matmuls x 5 k-tiles accumulated in PSUM;
     v kept SBUF-resident, q/k streamed to DRAM + fused squared-norm reduce
  C: DMA-transpose q/k chunks; per-head logits G accumulated over 128 chunks;
     norm/scale applied to G; softmax; FW = wproj @ blockdiag(attn)
  D: out = FW @ v -> f32 output
"""
import numpy as np
import ml_dtypes

import concourse.bass as bass
import concourse.tile as tile
from concourse import bacc, mybir
from concourse.bass_utils import run_bass_kernel_spmd

BF16NP = ml_dtypes.bfloat16
BF = mybir.dt.bfloat16
F32 = mybir.dt.float32

B, C, H, W = 8, 192, 128, 128
HEADS, CH = 4, 48
C3 = 3 * C                      # 576
HP, WP = H + 2, W + 2           # 130
NPIX = H * W                    # 16384
NPP = HP * WP                   # 16900
NT = 512
NTB = NPIX // NT                # 32 conv n-tiles
NTA = 34                        # stage-A n-tiles (33x512 + 1 overlapping)
KT = [(0, 128), (128, 128), (256, 128), (384, 128), (512, 64)]   # 576 split
KT2 = [(0, 128), (128, 64)]                                      # 192 split
MT_OUT = [(0, 128), (128, 64)]                                   # out-ch split

_CACHE = {}


def _build(variant="full"):
    nc = bacc.Bacc("TRN2", target_bir_lowering=False, debug=False, num_devices=8)
    xp_d = nc.dram_tensor("xp", [C, NPP], BF, kind="ExternalInput").ap()
    w1t_d = nc.dram_tensor("w1t", [C, C3], BF, kind="ExternalInput").ap()
    w2sb_d = nc.dram_tensor("w2sb", [5, 128, 9 * C3], BF, kind="ExternalInput").ap()
    wpt_d = nc.dram_tensor("wpt", [CH, HEADS * C], BF, kind="ExternalInput").ap()
    scale_d = nc.dram_tensor("scale", [1, HEADS], F32, kind="ExternalInput").ap()
    out_d = nc.dram_tensor("out", [C, NPIX], F32, kind="ExternalOutput").ap()

    with tile.TileContext(nc) as tc:
        with tc.tile_pool(name="dram", bufs=1, space="DRAM") as dram:
            y1p = dram.tile([C3, NPP], BF)
            qk = dram.tile([2 * C, NPIX], BF)
            rinv_d = dram.tile([1, 512], F32)
            fwt_d = dram.tile([C, C], BF)
            _build_body(nc, tc, xp_d, w1t_d, w2sb_d, wpt_d, scale_d, out_d,
                        y1p, qk, rinv_d, fwt_d, variant)
    nc.compile()
    return nc


def _build_body(nc, tc, xp_d, w1t_d, w2sb_d, wpt_d, scale_d, out_d,
                y1p, qk, rinv_d, fwt_d, variant="full"):
    mult = mybir.AluOpType.mult
    add = mybir.AluOpType.add

    with tc.tile_pool(name="persist", bufs=1) as persist:
        v0 = persist.tile([128, NPIX], BF, tag="v0")
        v1 = persist.tile([64, NPIX], BF, tag="v1")
        parts = [persist.tile([mp, NTB], F32, tag=f"part{i}", name=f"part{i}")
                 for i, (m0, mp) in enumerate(KT[:3])]

        # ---------------- Phase A + B ----------------
        with (tc.tile_pool(name="wts", bufs=1) as wts,
              tc.tile_pool(name="xk", bufs=4) as xkp,
              tc.tile_pool(name="slab", bufs=10) as slabp,
              tc.tile_pool(name="stage", bufs=6) as stagep,
              tc.tile_pool(name="sq", bufs=3) as sqp,
              tc.tile_pool(name="psA", bufs=4, space="PSUM") as psA):

            w1s = []
            for i, (k0, kp) in enumerate(KT2):
                t = wts.tile([kp, C3], BF, tag=f"w1_{i}")
                nc.sync.dma_start(t[:], w1t_d[k0:k0 + kp, :])
                w1s.append(t)
            w2s = []
            for i, (k0, kp) in enumerate(KT):
                t = wts.tile([kp, 9 * C3], BF, tag=f"w2_{i}")
                nc.sync.dma_start(t[:], w2sb_d[i, :kp, :])
                w2s.append(t)

            # Phase A: y1p = w1 @ xp
            for t in range(NTA):
                off = t * NT if t < NTA - 1 else NPP - NT
                xks = []
                for i, (k0, kp) in enumerate(KT2):
                    xk = xkp.tile([128, NT], BF, tag="xk")
                    nc.sync.dma_start(xk[:kp], xp_d[k0:k0 + kp, off:off + NT])
                    xks.append(xk)
                for (m0, mp) in KT:
                    ps = psA.tile([128, NT], F32, tag="ps")
                    for i, (k0, kp) in enumerate(KT2):
                        nc.tensor.matmul(ps[:mp], w1s[i][:, m0:m0 + mp],
                                         xks[i][:kp], start=(i == 0), stop=(i == 1))
                    st = stagep.tile([128, NT], BF, tag="stage")
                    nc.any.tensor_copy(st[:mp], ps[:mp])
                    nc.sync.dma_start(y1p[m0:m0 + mp, off:off + NT], st[:mp])

            # Phase B: conv3x3 via 9 shifted matmuls
            y1p_img = y1p.rearrange("c (h w) -> c h w", h=HP)
            for t in range(NTB):
                slabs = []
                for i, (k0, kp) in enumerate(KT):
                    sl = slabp.tile([128, 6, WP], BF, tag="slab")
                    nc.sync.dma_start(sl[:kp], y1p_img[k0:k0 + kp, 4 * t:4 * t + 6, :])
                    slabs.append(sl)
                for mi, (m0, mp) in enumerate(KT):
                    ps = psA.tile([128, NT], F32, tag="ps")
                    n_mm = 0
                    for s in range(9):
                        dy, dx = s // 3, s % 3
                        for i, (k0, kp) in enumerate(KT):
                            nc.tensor.matmul(
                                ps[:mp],
                                w2s[i][:, s * C3 + m0: s * C3 + m0 + mp],
                                slabs[i][:kp, dy:dy + 4, dx:dx + W],
                                start=(n_mm == 0), stop=(n_mm == 44))
                            n_mm += 1
                    if mi >= 3:   # v channels -> SBUF resident
                        vt = v0 if mi == 3 else v1
                        nc.any.tensor_copy(vt[:mp, t * NT:(t + 1) * NT], ps[:mp])
                        if variant == "ab":
                            stf = stagep.tile([128, NT], F32, tag="stagef",
                                              name="stf")
                            nc.any.tensor_copy(stf[:mp], ps[:mp])
                            nc.sync.dma_start(
                                out_d[m0 - 384:m0 - 384 + mp, t * NT:(t + 1) * NT],
                                stf[:mp])
                    else:         # q/k channels -> DRAM + norm partials
                        st = stagep.tile([128, NT], BF, tag="stage")
                        nc.any.tensor_copy(st[:mp], ps[:mp])
                        sq = sqp.tile([128, NT], F32, tag="sq")
                        nc.vector.tensor_mul(sq[:mp], st[:mp], st[:mp])
                        nc.vector.reduce_sum(parts[mi][:mp, t:t + 1], sq[:mp],
                                             axis=mybir.AxisListType.X)
                        nc.sync.dma_start(qk[m0:m0 + mp, t * NT:(t + 1) * NT], st[:mp])
        if variant == "ab":
            return

        # ---------------- Phase C + D ----------------
        petrans = (variant == "petrans")
        with (tc.tile_pool(name="qkt", bufs=1) as qktp,
              tc.tile_pool(name="small", bufs=1) as smallp,
              tc.tile_pool(name="soft", bufs=2) as softp,
              tc.tile_pool(name="ostage", bufs=4) as ostagep,
              tc.tile_pool(name="psG", bufs=1, space="PSUM") as psG,
              tc.tile_pool(name="psF", bufs=1 if petrans else 2, space="PSUM") as psF,
              tc.tile_pool(name="psD", bufs=1 if petrans else 2, space="PSUM") as psD):

            # finalize norms: rinv = 1/sqrt(ssq) -> DRAM (for head-aligned reload)
            for mi, (m0, mp) in enumerate(KT[:3]):
                ssq = smallp.tile([128, 1], F32, tag=f"ssq{mi}")
                nc.vector.reduce_sum(ssq[:mp], parts[mi][:mp, :], axis=mybir.AxisListType.X)
                nc.scalar.sqrt(ssq[:mp], ssq[:mp])
                nc.vector.reciprocal(ssq[:mp], ssq[:mp])
                nc.sync.dma_start(rinv_d[0, 128 * mi:128 * mi + mp], ssq[:mp, 0])

            # alpha[c,h] = scale[h] / ||q_{h,c}|| ; ball[c, 48h+d] = 1/||k_{h,d}||
            alpha = smallp.tile([CH, HEADS], F32, tag="alpha")
            for h in range(HEADS):
                nc.sync.dma_start(alpha[:, h:h + 1], rinv_d[0, CH * h:CH * (h + 1)])
            scs = smallp.tile([CH, HEADS], F32, tag="scs")
            nc.gpsimd.dma_start(out=scs[:], in_=scale_d[0:1, :].to_broadcast((CH, HEADS)))
            nc.vector.tensor_mul(alpha[:], alpha[:], scs[:])
            ball = smallp.tile([CH, C], F32, tag="ball")
            nc.gpsimd.dma_start(out=ball[:], in_=rinv_d[0:1, C:2 * C].to_broadcast((CH, C)))

            wpt_sb = smallp.tile([CH, HEADS * C], BF, tag="wpt")
            nc.sync.dma_start(wpt_sb[:], wpt_d[:, :])

            # logits: DMA-transpose qk chunks, accumulate G per head over chunks
            Gs = [psG.tile([CH, CH], F32, tag=f"G{h}", name=f"G{h}") for h in range(HEADS)]
            qkts = []
            if petrans:
                from concourse.masks import make_identity
                ident = smallp.tile([128, 128], BF, tag="ident")
                make_identity(nc, ident[:])
                with (tc.tile_pool(name="qksb", bufs=4) as qksbp,
                      tc.tile_pool(name="psT", bufs=2, space="PSUM") as psT):
                    for j in range(128):
                        qkt = qktp.tile([128, 2 * C], BF, tag=f"qkt{j}", name=f"qkt{j}")
                        for i, (c0, cp) in enumerate([(0, 128), (128, 128), (256, 128)]):
                            qksb = qksbp.tile([128, 128], BF, tag="qksb", name="qksb")
                            nc.sync.dma_start(qksb[:cp], qk[c0:c0 + cp, j * 128:(j + 1) * 128])
                            pst = psT.tile([128, 128], BF, tag="pst", name="pst")
                            nc.tensor.transpose(pst[:, :cp], qksb[:cp], ident[:cp, :cp])
                            nc.any.tensor_copy(qkt[:, i * 128:i * 128 + cp], pst[:, :cp])
                        qkts.append(qkt)
            else:
                for j in range(128):
                    qkt = qktp.tile([128, 2 * C], BF, tag=f"qkt{j}")
                    nc.sync.dma_start_transpose(qkt[:], qk[:, j * 128:(j + 1) * 128])
                    qkts.append(qkt)
            for j in range(128):
                for h in range(HEADS):
                    nc.tensor.matmul(
                        Gs[h][:], qkts[j][:, CH * h:CH * (h + 1)],
                        qkts[j][:, C + CH * h:C + CH * (h + 1)],
                        start=(j == 0), stop=(j == 127))

            # softmax + FW = wproj @ blockdiag(A)
            for h in range(HEADS):
                nc.vector.tensor_scalar_mul(Gs[h][:], Gs[h][:], alpha[:, h:h + 1])
                gsb = softp.tile([CH, CH], F32, tag="gsb")
                nc.vector.tensor_mul(gsb[:], Gs[h][:], ball[:, CH * h:CH * (h + 1)])
                mx = softp.tile([CH, 1], F32, tag="mx")
                nc.vector.reduce_max(mx[:], gsb[:], axis=mybir.AxisListType.X)
                nc.vector.tensor_scalar_mul(mx[:], mx[:], -1.0)
                ex = softp.tile([CH, CH], F32, tag="ex")
                nc.scalar.activation(ex[:], gsb[:], mybir.ActivationFunctionType.Exp,
                                     bias=mx[:], scale=1.0)
                sm = softp.tile([CH, 1], F32, tag="sm")
                nc.vector.reduce_sum(sm[:], ex[:], axis=mybir.AxisListType.X)
                nc.vector.reciprocal(sm[:], sm[:])
                asb = softp.tile([CH, CH], BF, tag="asb")
                nc.vector.tensor_scalar_mul(asb[:], ex[:], sm[:, 0:1])
                fw_ps = psF.tile([CH, C], F32, tag="fw")
                nc.tensor.matmul(fw_ps[:], asb[:], wpt_sb[:, C * h:C * (h + 1)],
                                 start=True, stop=True)
                fw_sb = softp.tile([CH, C], BF, tag="fwsb")
                nc.any.tensor_copy(fw_sb[:], fw_ps[:])
                nc.sync.dma_start(fwt_d[CH * h:CH * (h + 1), :], fw_sb[:])

            fwt0 = smallp.tile([128, C], BF, tag="fwt0")
            nc.sync.dma_start(fwt0[:], fwt_d[0:128, :])
            fwt1 = smallp.tile([64, C], BF, tag="fwt1")
            nc.sync.dma_start(fwt1[:], fwt_d[128:C, :])

            # Phase D: out = FW @ v
            for t in range(NTB):
                for (m0, mp) in MT_OUT:
                    ps = psD.tile([128, NT], F32, tag="psD")
                    nc.tensor.matmul(ps[:mp], fwt0[:, m0:m0 + mp],
                                     v0[:, t * NT:(t + 1) * NT], start=True, stop=False)
                    nc.tensor.matmul(ps[:mp], fwt1[:, m0:m0 + mp],
                                     v1[:, t * NT:(t + 1) * NT], start=False, stop=True)
                    ost = ostagep.tile([128, NT], F32, tag="ost")
                    nc.any.tensor_copy(ost[:mp], ps[:mp])
                    nc.sync.dma_start(out_d[m0:m0 + mp, t * NT:(t + 1) * NT], ost[:mp])


def _prep_shared(w_qkv1, w_qkv2, w_proj, scale):
    w1t = np.ascontiguousarray(w_qkv1[:, :, 0, 0].T).astype(BF16NP)       # [192,576]
    w2t = np.transpose(w_qkv2, (2, 3, 1, 0)).reshape(9, C3, C3)          # [s,i,o]
    w2sb = np.zeros((5, 128, 9 * C3), dtype=BF16NP)
    for kt, (k0, kp) in enumerate(KT):
        w2sb[kt, :kp, :] = np.ascontiguousarray(
            np.transpose(w2t[:, k0:k0 + kp, :], (1, 0, 2)).reshape(kp, 9 * C3)
        ).astype(BF16NP)
    wpf = w_proj[:, :, 0, 0].T                                            # [c,o]
    wpt = np.concatenate([wpf[h * CH:(h + 1) * CH, :] for h in range(HEADS)],
                         axis=1).astype(BF16NP)                           # [48,768]
    sc = np.asarray(scale, np.float32).reshape(1, HEADS)
    return w1t, w2sb, wpt, sc


def kernel(x, w_qkv1, w_qkv2, w_proj, scale):
    x = np.asarray(x, np.float32)
    if "nc" not in _CACHE:
        _CACHE["nc"] = _build()
    nc = _CACHE["nc"]

    w1t, w2sb, wpt, sc = _prep_shared(
        np.asarray(w_qkv1, np.float32), np.asarray(w_qkv2, np.float32),
        np.asarray(w_proj, np.float32), np.asarray(scale, np.float32))

    xp = np.zeros((B, C, HP, WP), np.float32)
    xp[:, :, 1:H + 1, 1:W + 1] = x
    xp = xp.astype(BF16NP).reshape(B, C, NPP)

    in_maps = [{"xp": xp[i], "w1t": w1t, "w2sb": w2sb, "wpt": wpt, "scale": sc}
               for i in range(B)]
    res = run_bass_kernel_spmd(nc, in_maps, core_ids=list(range(B)))
    out = np.stack([res.results[i]["out"].reshape(C, H, W) for i in range(B)], 0)
    return np.ascontiguousarray(out.astype(np.float32))
